# revision 11
# baseline (speedup 1.0000x reference)
"""Trainium2 Bass kernel for causal self-attention (B=4, T=2048, D=1024, H=16).

Sharding: 8 cores = 4 batches x 2 query-shards. Each core computes, for its
batch, the full K/V projection (prefix recompute instead of collectives), the
Q projection for its 8 query blocks of 128 rows, causal attention for all 16
heads over those query blocks, and the output projection for its rows. Query
blocks are interleaved between the two cores of a batch ({0,2,4,6,9,11,13,15}
vs {1,3,5,7,8,10,12,14}) and padded to a uniform causal-length schedule
(slot j covers 2*(j+1) key chunks), so all 8 cores run one identical program
on different data. Host gathers the disjoint output slabs - no collectives.

All matmuls run in bf16 with fp32 PSUM accumulation (verified ~3.5e-3
rel-to-absmax vs the fp32 reference). x is sent pre-transposed from the host
so no on-device transposes of the activations are needed.

Attention is software-pipelined by head pair: the QK+exp stage of pair hp is
interleaved at chunk-group/slot granularity with the AV+normalize stage of
pair hp-1, keeping the PE dense so the HAM clock gate stays at full rate.
Head pairs share the 128-partition PE array via row groups (even head rows
0-63, odd head rows 64-127). Matmul PSUM writes never cross a 512-fp32 bank
boundary (hardware requirement). The output projection of slot j is emitted
right after the final pair finishes slot j, hiding the tail.
"""

import numpy as np
import ml_dtypes

import concourse.bass as bass
import concourse.tile as tile
from concourse import mybir
from concourse.bass_utils import run_bass_kernel_spmd
from concourse.masks import make_identity

P = 128
T = 2048
D = 1024
H = 16
DH = 64
NSLOT = 8          # query blocks per core
NCH = 16           # key chunks of 128
BF16 = mybir.dt.bfloat16
F32 = mybir.dt.float32

# slot j processes key chunks 0 .. PL[j]-1 (uniform padded causal schedule)
PL = [2 * (j + 1) for j in range(NSLOT)]
# key chunk c feeds query slots jmin(c)..7, jmin = c//2
NCOLS = [(NSLOT - c // 2) * P for c in range(NCH)]
OFF = np.cumsum([0] + NCOLS).tolist()          # exp-buffer offsets, total 9216
# chunk groups whose logits fit one [128, 1024] PSUM tile -> one exp call each
CGROUPS = [[0], [1], [2], [3], [4], [5], [6], [7], [8, 9], [10, 11], [12, 13], [14, 15]]

# global query-block indices per role (core parity)
GBLOCKS = {
    0: [0, 2, 4, 6, 9, 11, 13, 15],
    1: [1, 3, 5, 7, 8, 10, 12, 14],
}

_CACHED_NC = None
LAST_RESULTS = None


def _build_nc():
    nc = bass.Bass()
    x_kvT = nc.declare_dram_parameter("x_kvT", [D, T], BF16, isOutput=False)
    x_qT = nc.declare_dram_parameter("x_qT", [D, NSLOT * P], BF16, isOutput=False)
    w_qkv = nc.declare_dram_parameter("w_qkv", [D, 3 * D], BF16, isOutput=False)
    w_out = nc.declare_dram_parameter("w_out", [D, D], BF16, isOutput=False)
    mask = nc.declare_dram_parameter("mask", [P, NCH, P], BF16, isOutput=False)
    out = nc.declare_dram_parameter("out", [NSLOT * P, D], F32, isOutput=True)

    with tile.TileContext(nc) as tc:
        with tc.tile_pool(name="persist", bufs=1) as pp:
            ident = pp.tile([P, P], BF16)
            make_identity(nc, ident[:])
            kT = pp.tile([P, 8, T], BF16)            # K^T, e-dims on partitions
            qT = pp.tile([P, 8, NSLOT * P], BF16)    # Q^T
            vA = pp.tile([P, NCH, H, DH + 1], BF16)  # V with ones column per head
            yT = pp.tile([P, 8, NSLOT * P], BF16)    # normalized attn out, transposed
            msk = pp.tile([P, NCH, P], BF16)
            nc.gpsimd.memset(vA[:, :, :, DH], 1.0)

            # ---------------- phase 1: QKV projections ------------------------
            with (
                tc.tile_pool(name="ph1", bufs=1) as p1,
                tc.tile_pool(name="wq_pool", bufs=3) as wqp,
                tc.tile_pool(name="wv_pool", bufs=2) as wvp,
                tc.tile_pool(name="ppsum", bufs=4, space="PSUM") as pps,
            ):
                xqTs = p1.tile([P, 8, NSLOT * P], BF16)
                xkTs = p1.tile([P, 8, T], BF16)
                xq_r = x_qT.rearrange("(dc p) t -> p dc t", p=P)
                xk_r = x_kvT.rearrange("(dc p) t -> p dc t", p=P)
                for nt in range(2):
                    nc.sync.dma_start(xqTs[:, :, nt * 512:(nt + 1) * 512],
                                      xq_r[:, :, nt * 512:(nt + 1) * 512])
                for nt in range(4):
                    nc.sync.dma_start(xkTs[:, :, nt * 512:(nt + 1) * 512],
                                      xk_r[:, :, nt * 512:(nt + 1) * 512])

                # Q^T: out[e_tile, q] = sum_d w_q[d, e]^T x_q[d, q]
                for et in range(8):
                    wt = wqp.tile([P, 8, P], BF16, tag="wq")
                    nc.sync.dma_start(
                        wt[:],
                        w_qkv[:, et * P:(et + 1) * P].rearrange("(dc p) e -> p dc e", p=P),
                    )
                    for nt in range(2):
                        ps = pps.tile([P, 512], F32, tag="ps")
                        for dc in range(8):
                            nc.tensor.matmul(
                                ps[:], wt[:, dc, :], xqTs[:, dc, nt * 512:(nt + 1) * 512],
                                start=(dc == 0), stop=(dc == 7),
                            )
                        nc.scalar.copy(out=qT[:, et, nt * 512:(nt + 1) * 512], in_=ps[:])

                # K^T over full 2048 keys
                for et in range(8):
                    wt = wqp.tile([P, 8, P], BF16, tag="wq")
                    nc.sync.dma_start(
                        wt[:],
                        w_qkv[:, D + et * P:D + (et + 1) * P].rearrange("(dc p) e -> p dc e", p=P),
                    )
                    for nt in range(4):
                        ps = pps.tile([P, 512], F32, tag="ps")
                        for dc in range(8):
                            nc.tensor.matmul(
                                ps[:], wt[:, dc, :], xkTs[:, dc, nt * 512:(nt + 1) * 512],
                                start=(dc == 0), stop=(dc == 7),
                            )
                        nc.scalar.copy(out=kT[:, et, nt * 512:(nt + 1) * 512], in_=ps[:])

                # V in natural [t, e] layout, interleaved with the ones column
                for nt in range(2):
                    wv = wvp.tile([P, 8, 512], BF16, tag="wv")
                    nc.sync.dma_start(
                        wv[:],
                        w_qkv[:, 2 * D + nt * 512:2 * D + (nt + 1) * 512].rearrange(
                            "(dc p) e -> p dc e", p=P),
                    )
                    for tt in range(NCH):
                        ps = pps.tile([P, 512], F32, tag="ps")
                        for dc in range(8):
                            nc.tensor.matmul(
                                ps[:], xkTs[:, dc, tt * P:(tt + 1) * P], wv[:, dc, :],
                                start=(dc == 0), stop=(dc == 7),
                            )
                        nc.scalar.copy(
                            out=vA[:, tt, nt * 8:(nt + 1) * 8, 0:DH],
                            in_=ps.rearrange("p (h d) -> p h d", d=DH),
                        )

            nc.sync.dma_start(msk[:], mask[:])

            # ---------------- phase 2: attention ------------------------------
            with (
                tc.tile_pool(name="norm_pool", bufs=4) as np_,
                tc.tile_pool(name="spsum", bufs=4, space="PSUM") as sps,
                tc.tile_pool(name="exp_b", bufs=2) as ep_b,
            ):
                lps = None
                wo = None
                ops = None
                obp = None

                def emit_qk_group(hp, ehs, grp):
                    """One chunk group of QK + exp + mask for head pair hp."""
                    et = hp
                    for r0, eh in ((0, ehs[0]), (64, ehs[1])):
                        lp = lps.tile([P, 1024], F32, tag="lp")
                        pos = 0
                        for c in grp:
                            jm = c // 2
                            ncols = NCOLS[c]
                            s = 0
                            while s < ncols:
                                # a matmul PSUM write must not cross a bank
                                # boundary (512 fp32 per bank)
                                w_ = min(512, ncols - s, 512 - (pos + s) % 512)
                                nc.tensor.matmul(
                                    lp[:, pos + s:pos + s + w_],
                                    kT[r0:r0 + 64, et, c * P:(c + 1) * P],
                                    qT[r0:r0 + 64, et, jm * P + s:jm * P + s + w_],
                                    start=True, stop=True,
                                )
                                s += w_
                            pos += ncols
                        nc.scalar.activation(
                            eh[:, OFF[grp[0]]:OFF[grp[0]] + pos], lp[:, :pos],
                            mybir.ActivationFunctionType.Exp, scale=0.125,
                        )
                        for c in grp:
                            # causal/padding mask on the diagonal slot of chunk c
                            nc.gpsimd.tensor_mul(
                                out=eh[:, OFF[c]:OFF[c] + P],
                                in0=eh[:, OFF[c]:OFF[c] + P],
                                in1=msk[:, c, :],
                            )

                def emit_out_slot(j):
                    op = ops.tile([P, D], F32, tag="op")
                    for nh in range(2):
                        for kc in range(8):
                            nc.tensor.matmul(
                                op[:, nh * 512:(nh + 1) * 512],
                                yT[:, kc, j * P:(j + 1) * P],
                                wo[:, kc, nh * 512:(nh + 1) * 512],
                                start=(kc == 0), stop=(kc == 7),
                            )
                    ob = obp.tile([P, D], F32, tag="ob")
                    nc.vector.tensor_copy(out=ob[:], in_=op[:])
                    nc.sync.dma_start(out[j * P:(j + 1) * P, :], ob[:])

                def emit_av_slot(hp, ehs, j, emit_out=False):
                    """AV + normalize + transpose for slot j of head pair hp."""
                    et = hp
                    yn2 = np_.tile([P, P], BF16, tag="yn2")
                    for parity, eh in enumerate(ehs):
                        h = 2 * hp + parity
                        ya = sps.tile([P, DH + 1], F32, tag="small",
                                      name=f"ya{hp}_{j}_{parity}")
                        for c in range(PL[j]):
                            jm = c // 2
                            nc.tensor.matmul(
                                ya[:],
                                eh[:, OFF[c] + (j - jm) * P:OFF[c] + (j - jm + 1) * P],
                                vA[:, c, h, :],
                                start=(c == 0), stop=(c == PL[j] - 1),
                            )
                        rec = np_.tile([P, 1], F32, tag="rec")
                        nc.vector.reciprocal(rec[:], ya[:, DH:DH + 1])
                        nc.vector.tensor_scalar_mul(
                            yn2[:, parity * DH:(parity + 1) * DH], ya[:, 0:DH], rec[:])
                    yt2 = sps.tile([P, P], BF16, tag="small", name=f"yt{hp}_{j}")
                    nc.tensor.transpose(yt2[:], yn2[:], ident[:])
                    nc.vector.tensor_copy(out=yT[:, et, j * P:(j + 1) * P], in_=yt2[:])
                    if emit_out:
                        emit_out_slot(j)

                prev = None
                with (
                    tc.tile_pool(name="exp_a", bufs=2) as ep_a,
                    tc.tile_pool(name="lpsum", bufs=2, space="PSUM") as lps_,
                ):
                    lps = lps_
                    for hp in range(H // 2):
                        pool = ep_a if hp % 2 == 0 else ep_b
                        ehs = (pool.tile([P, OFF[NCH]], BF16, tag="exph", name=f"eh{hp}a"),
                               pool.tile([P, OFF[NCH]], BF16, tag="exph", name=f"eh{hp}b"))
                        # interleave this pair's QK groups with the previous
                        # pair's AV slots (longest slots first) so the PE
                        # always has independent work queued
                        av_js = list(range(NSLOT - 1, -1, -1))
                        for i, grp in enumerate(CGROUPS):
                            emit_qk_group(hp, ehs, grp)
                            if prev is not None and i < NSLOT:
                                emit_av_slot(hp - 1, prev, av_js[i])
                        prev = ehs

                # last pair's AV, fused with the output projection
                with (
                    tc.tile_pool(name="wo_pool", bufs=1) as wop,
                    tc.tile_pool(name="ob_pool", bufs=2) as obp_,
                    tc.tile_pool(name="opsum", bufs=2, space="PSUM") as ops_,
                ):
                    obp = obp_
                    ops = ops_
                    wo = wop.tile([P, 8, D], BF16)
                    for dc in range(8):
                        nc.sync.dma_start(wo[:, dc, :], w_out[dc * P:(dc + 1) * P, :])
                    for j in range(NSLOT):
                        emit_av_slot(H // 2 - 1, prev, j, emit_out=True)

    _split_waits(nc, 1)
    return nc


def _split_waits(nc, maxw=1):
    """walrus rejects instructions with more than one sync wait; hoist extra
    waits onto preceding same-engine Drain instructions."""
    nsplit = 0
    for f in nc.m.functions:
        for b in f.blocks:
            insts = b.instructions
            new = []
            changed = False
            for inst in insts:
                si = inst.sync_info
                if si is not None and len(si.on_wait) > maxw:
                    waits = list(si.on_wait)
                    chunks = [waits[i:i + maxw] for i in range(0, len(waits), maxw)]
                    for ci, ch in enumerate(chunks[:-1]):
                        d = mybir.InstDrain(name=f"{inst.name}-wsplit{ci}", ins=[], outs=[])
                        d.engine = inst.engine
                        d.sync_info = mybir.SyncInfo(on_wait=ch, on_update=[])
                        new.append(d)
                        nsplit += 1
                    inst.sync_info = mybir.SyncInfo(
                        on_wait=chunks[-1], on_update=list(si.on_update))
                    changed = True
                new.append(inst)
            if changed:
                b.instructions = new
    return nsplit


def _host_mask(role):
    g = GBLOCKS[role]
    m = np.zeros((P, NCH, P), np.float32)
    for c in range(NCH):
        j = c // 2
        kk = c * P + np.arange(P)[:, None]       # global key index
        qq = g[j] * P + np.arange(P)[None, :]    # global query index
        m[:, c, :] = (kk <= qq).astype(np.float32)
    return m.astype(ml_dtypes.bfloat16)


def kernel(x, w_qkv, w_out):
    global _CACHED_NC, LAST_RESULTS
    x = np.asarray(x)
    w_qkv = np.asarray(w_qkv)
    w_out = np.asarray(w_out)
    B = x.shape[0]
    assert x.shape == (B, T, D) and B * 2 == 8

    if _CACHED_NC is None:
        _CACHED_NC = _build_nc()
    nc = _CACHED_NC

    wq_b = w_qkv.astype(ml_dtypes.bfloat16)
    wo_b = w_out.astype(ml_dtypes.bfloat16)
    masks = {r: _host_mask(r) for r in (0, 1)}

    in_maps = []
    for core in range(8):
        b, role = divmod(core, 2)
        xb = x[b].astype(ml_dtypes.bfloat16)
        g = GBLOCKS[role]
        xq = np.concatenate([xb[gi * P:(gi + 1) * P] for gi in g], axis=0)
        in_maps.append({
            "x_kvT": np.ascontiguousarray(xb.T),
            "x_qT": np.ascontiguousarray(xq.T),
            "w_qkv": wq_b,
            "w_out": wo_b,
            "mask": masks[role],
        })

    res = run_bass_kernel_spmd(nc, in_maps, core_ids=list(range(8)))
    LAST_RESULTS = res

    y = np.empty((B, T, D), np.float32)
    for core in range(8):
        b, role = divmod(core, 2)
        slab = res.results[core]["out"]
        g = GBLOCKS[role]
        for j, gi in enumerate(g):
            y[b, gi * P:(gi + 1) * P, :] = slab[j * P:(j + 1) * P, :]
    return y


# revision 12
# speedup vs baseline: 1.0062x; 1.0062x over previous
"""Trainium2 Bass kernel for causal self-attention (B=4, T=2048, D=1024, H=16).

Sharding: 8 cores = 4 batches x 2 query-shards. Each core computes, for its
batch, the full K/V projection (prefix recompute instead of collectives), the
Q projection for its 8 query blocks of 128 rows, causal attention for all 16
heads over those query blocks, and the output projection for its rows. Query
blocks are interleaved between the two cores of a batch ({0,2,4,6,9,11,13,15}
vs {1,3,5,7,8,10,12,14}) and padded to a uniform causal-length schedule
(slot j covers 2*(j+1) key chunks), so all 8 cores run one identical program
on different data. Host gathers the disjoint output slabs - no collectives.

All matmuls run in bf16 with fp32 PSUM accumulation (verified ~3.5e-3
rel-to-absmax vs the fp32 reference). x is sent pre-transposed from the host
so no on-device transposes of the activations are needed.

Attention is software-pipelined by head pair: the QK+exp stage of pair hp is
interleaved at chunk-group/slot granularity with the AV+normalize stage of
pair hp-1, keeping the PE dense so the HAM clock gate stays at full rate.
Head pairs share the 128-partition PE array via row groups (even head rows
0-63, odd head rows 64-127). Matmul PSUM writes never cross a 512-fp32 bank
boundary (hardware requirement). The output projection of slot j is emitted
right after the final pair finishes slot j, hiding the tail.
"""

import numpy as np
import ml_dtypes

import concourse.bass as bass
import concourse.tile as tile
from concourse import mybir
from concourse.bass_utils import run_bass_kernel_spmd
from concourse.masks import make_identity

P = 128
T = 2048
D = 1024
H = 16
DH = 64
NSLOT = 8          # query blocks per core
NCH = 16           # key chunks of 128
BF16 = mybir.dt.bfloat16
F32 = mybir.dt.float32

# slot j processes key chunks 0 .. PL[j]-1 (uniform padded causal schedule)
PL = [2 * (j + 1) for j in range(NSLOT)]
# key chunk c feeds query slots jmin(c)..7, jmin = c//2
NCOLS = [(NSLOT - c // 2) * P for c in range(NCH)]
OFF = np.cumsum([0] + NCOLS).tolist()          # exp-buffer offsets, total 9216
# chunk groups whose logits fit one [128, 1024] PSUM tile -> one exp call each
CGROUPS = [[0], [1], [2], [3], [4], [5], [6], [7], [8, 9], [10, 11], [12, 13], [14, 15]]

# global query-block indices per role (core parity)
GBLOCKS = {
    0: [0, 2, 4, 6, 9, 11, 13, 15],
    1: [1, 3, 5, 7, 8, 10, 12, 14],
}

_CACHED_NC = None
LAST_RESULTS = None


def _build_nc():
    nc = bass.Bass()
    x_kvT = nc.declare_dram_parameter("x_kvT", [D, T], BF16, isOutput=False)
    x_qT = nc.declare_dram_parameter("x_qT", [D, NSLOT * P], BF16, isOutput=False)
    w_qkv = nc.declare_dram_parameter("w_qkv", [D, 3 * D], BF16, isOutput=False)
    w_out = nc.declare_dram_parameter("w_out", [D, D], BF16, isOutput=False)
    mask = nc.declare_dram_parameter("mask", [P, NCH, P], BF16, isOutput=False)
    out = nc.declare_dram_parameter("out", [NSLOT * P, D], F32, isOutput=True)

    with tile.TileContext(nc) as tc:
        with tc.tile_pool(name="persist", bufs=1) as pp:
            ident = pp.tile([P, P], BF16)
            make_identity(nc, ident[:])
            kT = pp.tile([P, 8, T], BF16)            # K^T, e-dims on partitions
            qT = pp.tile([P, 8, NSLOT * P], BF16)    # Q^T
            vA = pp.tile([P, NCH, H, DH + 1], BF16)  # V with ones column per head
            yT = pp.tile([P, 8, NSLOT * P], BF16)    # normalized attn out, transposed
            msk = pp.tile([P, NCH, P], BF16)
            nc.gpsimd.memset(vA[:, :, :, DH], 1.0)

            # ---------------- phase 1: QKV projections ------------------------
            with (
                tc.tile_pool(name="ph1", bufs=1) as p1,
                tc.tile_pool(name="wq_pool", bufs=3) as wqp,
                tc.tile_pool(name="wv_pool", bufs=2) as wvp,
                tc.tile_pool(name="ppsum", bufs=4, space="PSUM") as pps,
            ):
                xqTs = p1.tile([P, 8, NSLOT * P], BF16)
                xkTs = p1.tile([P, 8, T], BF16)
                xq_r = x_qT.rearrange("(dc p) t -> p dc t", p=P)
                xk_r = x_kvT.rearrange("(dc p) t -> p dc t", p=P)
                for nt in range(2):
                    nc.sync.dma_start(xqTs[:, :, nt * 512:(nt + 1) * 512],
                                      xq_r[:, :, nt * 512:(nt + 1) * 512])
                for nt in range(4):
                    nc.sync.dma_start(xkTs[:, :, nt * 512:(nt + 1) * 512],
                                      xk_r[:, :, nt * 512:(nt + 1) * 512])

                # Q^T: out[e_tile, q] = sum_d w_q[d, e]^T x_q[d, q]
                for et in range(8):
                    wt = wqp.tile([P, 8, P], BF16, tag="wq")
                    nc.sync.dma_start(
                        wt[:],
                        w_qkv[:, et * P:(et + 1) * P].rearrange("(dc p) e -> p dc e", p=P),
                    )
                    for nt in range(2):
                        ps = pps.tile([P, 512], F32, tag="ps")
                        for dc in range(8):
                            nc.tensor.matmul(
                                ps[:], wt[:, dc, :], xqTs[:, dc, nt * 512:(nt + 1) * 512],
                                start=(dc == 0), stop=(dc == 7),
                            )
                        nc.scalar.copy(out=qT[:, et, nt * 512:(nt + 1) * 512], in_=ps[:])

                # K^T over full 2048 keys
                for et in range(8):
                    wt = wqp.tile([P, 8, P], BF16, tag="wq")
                    nc.sync.dma_start(
                        wt[:],
                        w_qkv[:, D + et * P:D + (et + 1) * P].rearrange("(dc p) e -> p dc e", p=P),
                    )
                    for nt in range(4):
                        ps = pps.tile([P, 512], F32, tag="ps")
                        for dc in range(8):
                            nc.tensor.matmul(
                                ps[:], wt[:, dc, :], xkTs[:, dc, nt * 512:(nt + 1) * 512],
                                start=(dc == 0), stop=(dc == 7),
                            )
                        nc.scalar.copy(out=kT[:, et, nt * 512:(nt + 1) * 512], in_=ps[:])

                # V in natural [t, e] layout, interleaved with the ones column
                for nt in range(2):
                    wv = wvp.tile([P, 8, 512], BF16, tag="wv")
                    nc.sync.dma_start(
                        wv[:],
                        w_qkv[:, 2 * D + nt * 512:2 * D + (nt + 1) * 512].rearrange(
                            "(dc p) e -> p dc e", p=P),
                    )
                    for tt in range(NCH):
                        ps = pps.tile([P, 512], F32, tag="ps")
                        for dc in range(8):
                            nc.tensor.matmul(
                                ps[:], xkTs[:, dc, tt * P:(tt + 1) * P], wv[:, dc, :],
                                start=(dc == 0), stop=(dc == 7),
                            )
                        nc.scalar.copy(
                            out=vA[:, tt, nt * 8:(nt + 1) * 8, 0:DH],
                            in_=ps.rearrange("p (h d) -> p h d", d=DH),
                        )

            nc.sync.dma_start(msk[:], mask[:])

            # ---------------- phase 2: attention ------------------------------
            with (
                tc.tile_pool(name="norm_pool", bufs=4) as np_,
                tc.tile_pool(name="spsum", bufs=4, space="PSUM") as sps,
                tc.tile_pool(name="exp_b", bufs=2) as ep_b,
            ):
                lps = None
                wo = None
                ops = None
                obp = None

                def emit_qk_group(hp, ehs, grp):
                    """One chunk group of QK + exp + mask for head pair hp."""
                    et = hp
                    for r0, eh in ((0, ehs[0]), (64, ehs[1])):
                        lp = lps.tile([P, 1024], F32, tag="lp")
                        pos = 0
                        for c in grp:
                            jm = c // 2
                            ncols = NCOLS[c]
                            s = 0
                            while s < ncols:
                                # a matmul PSUM write must not cross a bank
                                # boundary (512 fp32 per bank)
                                w_ = min(512, ncols - s, 512 - (pos + s) % 512)
                                nc.tensor.matmul(
                                    lp[:, pos + s:pos + s + w_],
                                    kT[r0:r0 + 64, et, c * P:(c + 1) * P],
                                    qT[r0:r0 + 64, et, jm * P + s:jm * P + s + w_],
                                    start=True, stop=True,
                                )
                                s += w_
                            pos += ncols
                        nc.scalar.activation(
                            eh[:, OFF[grp[0]]:OFF[grp[0]] + pos], lp[:, :pos],
                            mybir.ActivationFunctionType.Exp, scale=0.125,
                        )
                        for c in grp:
                            # causal/padding mask on the diagonal slot of chunk c
                            nc.vector.tensor_mul(
                                out=eh[:, OFF[c]:OFF[c] + P],
                                in0=eh[:, OFF[c]:OFF[c] + P],
                                in1=msk[:, c, :],
                            )

                def emit_out_slot(j):
                    op = ops.tile([P, D], F32, tag="op")
                    for nh in range(2):
                        for kc in range(8):
                            nc.tensor.matmul(
                                op[:, nh * 512:(nh + 1) * 512],
                                yT[:, kc, j * P:(j + 1) * P],
                                wo[:, kc, nh * 512:(nh + 1) * 512],
                                start=(kc == 0), stop=(kc == 7),
                            )
                    ob = obp.tile([P, D], F32, tag="ob")
                    nc.vector.tensor_copy(out=ob[:], in_=op[:])
                    nc.sync.dma_start(out[j * P:(j + 1) * P, :], ob[:])

                pending = []   # deferred (et, j, yn2, emit_out) transposes

                def flush_pending():
                    """Transpose+store slots whose normalize finished a while
                    ago - deferring these keeps the PE from stalling on the
                    DVE recip/scale chain right after each slot's AV."""
                    for et, j, yn2, do_out in pending:
                        yt2 = sps.tile([P, P], BF16, tag="small", name=f"yt{et}_{j}")
                        nc.tensor.transpose(yt2[:], yn2[:], ident[:])
                        nc.vector.tensor_copy(out=yT[:, et, j * P:(j + 1) * P], in_=yt2[:])
                        if do_out:
                            emit_out_slot(j)
                    pending.clear()

                def emit_av_slot(hp, ehs, j, emit_out=False):
                    """AV + normalize for slot j of head pair hp."""
                    et = hp
                    yn2 = np_.tile([P, P], BF16, tag="yn2")
                    for parity, eh in enumerate(ehs):
                        h = 2 * hp + parity
                        ya = sps.tile([P, DH + 1], F32, tag="small",
                                      name=f"ya{hp}_{j}_{parity}")
                        for c in range(PL[j]):
                            jm = c // 2
                            nc.tensor.matmul(
                                ya[:],
                                eh[:, OFF[c] + (j - jm) * P:OFF[c] + (j - jm + 1) * P],
                                vA[:, c, h, :],
                                start=(c == 0), stop=(c == PL[j] - 1),
                            )
                        rec = np_.tile([P, 1], F32, tag="rec")
                        nc.vector.reciprocal(rec[:], ya[:, DH:DH + 1])
                        nc.vector.tensor_scalar_mul(
                            yn2[:, parity * DH:(parity + 1) * DH], ya[:, 0:DH], rec[:])
                    flush_pending()
                    pending.append((et, j, yn2, emit_out))

                prev = None
                with (
                    tc.tile_pool(name="exp_a", bufs=2) as ep_a,
                    tc.tile_pool(name="lpsum", bufs=2, space="PSUM") as lps_,
                ):
                    lps = lps_
                    for hp in range(H // 2):
                        pool = ep_a if hp % 2 == 0 else ep_b
                        ehs = (pool.tile([P, OFF[NCH]], BF16, tag="exph", name=f"eh{hp}a"),
                               pool.tile([P, OFF[NCH]], BF16, tag="exph", name=f"eh{hp}b"))
                        # interleave this pair's QK groups with the previous
                        # pair's AV slots (longest slots first) so the PE
                        # always has independent work queued
                        av_js = list(range(NSLOT - 1, -1, -1))
                        for i, grp in enumerate(CGROUPS):
                            emit_qk_group(hp, ehs, grp)
                            if prev is not None and i < NSLOT:
                                emit_av_slot(hp - 1, prev, av_js[i])
                        prev = ehs

                # last pair's AV, fused with the output projection
                with (
                    tc.tile_pool(name="wo_pool", bufs=1) as wop,
                    tc.tile_pool(name="ob_pool", bufs=2) as obp_,
                    tc.tile_pool(name="opsum", bufs=2, space="PSUM") as ops_,
                ):
                    obp = obp_
                    ops = ops_
                    wo = wop.tile([P, 8, D], BF16)
                    for dc in range(8):
                        nc.sync.dma_start(wo[:, dc, :], w_out[dc * P:(dc + 1) * P, :])
                    for j in range(NSLOT):
                        emit_av_slot(H // 2 - 1, prev, j, emit_out=True)
                    flush_pending()

    _split_waits(nc, 1)
    return nc


def _split_waits(nc, maxw=1):
    """walrus rejects instructions with more than one sync wait; hoist extra
    waits onto preceding same-engine Drain instructions."""
    nsplit = 0
    for f in nc.m.functions:
        for b in f.blocks:
            insts = b.instructions
            new = []
            changed = False
            for inst in insts:
                si = inst.sync_info
                if si is not None and len(si.on_wait) > maxw:
                    waits = list(si.on_wait)
                    chunks = [waits[i:i + maxw] for i in range(0, len(waits), maxw)]
                    for ci, ch in enumerate(chunks[:-1]):
                        d = mybir.InstDrain(name=f"{inst.name}-wsplit{ci}", ins=[], outs=[])
                        d.engine = inst.engine
                        d.sync_info = mybir.SyncInfo(on_wait=ch, on_update=[])
                        new.append(d)
                        nsplit += 1
                    inst.sync_info = mybir.SyncInfo(
                        on_wait=chunks[-1], on_update=list(si.on_update))
                    changed = True
                new.append(inst)
            if changed:
                b.instructions = new
    return nsplit


def _host_mask(role):
    g = GBLOCKS[role]
    m = np.zeros((P, NCH, P), np.float32)
    for c in range(NCH):
        j = c // 2
        kk = c * P + np.arange(P)[:, None]       # global key index
        qq = g[j] * P + np.arange(P)[None, :]    # global query index
        m[:, c, :] = (kk <= qq).astype(np.float32)
    return m.astype(ml_dtypes.bfloat16)


def kernel(x, w_qkv, w_out):
    global _CACHED_NC, LAST_RESULTS
    x = np.asarray(x)
    w_qkv = np.asarray(w_qkv)
    w_out = np.asarray(w_out)
    B = x.shape[0]
    assert x.shape == (B, T, D) and B * 2 == 8

    if _CACHED_NC is None:
        _CACHED_NC = _build_nc()
    nc = _CACHED_NC

    wq_b = w_qkv.astype(ml_dtypes.bfloat16)
    wo_b = w_out.astype(ml_dtypes.bfloat16)
    masks = {r: _host_mask(r) for r in (0, 1)}

    in_maps = []
    for core in range(8):
        b, role = divmod(core, 2)
        xb = x[b].astype(ml_dtypes.bfloat16)
        g = GBLOCKS[role]
        xq = np.concatenate([xb[gi * P:(gi + 1) * P] for gi in g], axis=0)
        in_maps.append({
            "x_kvT": np.ascontiguousarray(xb.T),
            "x_qT": np.ascontiguousarray(xq.T),
            "w_qkv": wq_b,
            "w_out": wo_b,
            "mask": masks[role],
        })

    res = run_bass_kernel_spmd(nc, in_maps, core_ids=list(range(8)))
    LAST_RESULTS = res

    y = np.empty((B, T, D), np.float32)
    for core in range(8):
        b, role = divmod(core, 2)
        slab = res.results[core]["out"]
        g = GBLOCKS[role]
        for j, gi in enumerate(g):
            y[b, gi * P:(gi + 1) * P, :] = slab[j * P:(j + 1) * P, :]
    return y


# revision 13
# speedup vs baseline: 1.0098x; 1.0035x over previous
"""Trainium2 Bass kernel for causal self-attention (B=4, T=2048, D=1024, H=16).

Sharding: 8 cores = 4 batches x 2 query-shards. Each core computes, for its
batch, the full K/V projection (prefix recompute instead of collectives), the
Q projection for its 8 query blocks of 128 rows, causal attention for all 16
heads over those query blocks, and the output projection for its rows. Query
blocks are interleaved between the two cores of a batch ({0,2,4,6,9,11,13,15}
vs {1,3,5,7,8,10,12,14}) and padded to a uniform causal-length schedule
(slot j covers 2*(j+1) key chunks), so all 8 cores run one identical program
on different data. Host gathers the disjoint output slabs - no collectives.

All matmuls run in bf16 with fp32 PSUM accumulation (verified ~3.5e-3
rel-to-absmax vs the fp32 reference). x is sent pre-transposed from the host
so no on-device transposes of the activations are needed.

Attention is software-pipelined by head pair: the QK+exp stage of pair hp is
interleaved at chunk-group/slot granularity with the AV+normalize stage of
pair hp-1, keeping the PE dense so the HAM clock gate stays at full rate.
Head pairs share the 128-partition PE array via row groups (even head rows
0-63, odd head rows 64-127). Matmul PSUM writes never cross a 512-fp32 bank
boundary (hardware requirement). The output projection of slot j is emitted
right after the final pair finishes slot j, hiding the tail.
"""

import numpy as np
import ml_dtypes

import concourse.bass as bass
import concourse.tile as tile
from concourse import mybir
from concourse.bass_utils import run_bass_kernel_spmd
from concourse.masks import make_identity

P = 128
T = 2048
D = 1024
H = 16
DH = 64
NSLOT = 8          # query blocks per core
NCH = 16           # key chunks of 128
BF16 = mybir.dt.bfloat16
F32 = mybir.dt.float32

# slot j processes key chunks 0 .. PL[j]-1 (uniform padded causal schedule)
PL = [2 * (j + 1) for j in range(NSLOT)]
# key chunk c feeds query slots jmin(c)..7, jmin = c//2
NCOLS = [(NSLOT - c // 2) * P for c in range(NCH)]
OFF = np.cumsum([0] + NCOLS).tolist()          # exp-buffer offsets, total 9216
# chunk groups whose logits fit one [128, 1024] PSUM tile -> one exp call each
CGROUPS = [[0], [1], [2], [3], [4], [5], [6], [7], [8, 9], [10, 11], [12, 13], [14, 15]]

# global query-block indices per role (core parity)
GBLOCKS = {
    0: [0, 2, 4, 6, 9, 11, 13, 15],
    1: [1, 3, 5, 7, 8, 10, 12, 14],
}

_CACHED_NC = None
LAST_RESULTS = None


def _build_nc():
    nc = bass.Bass()
    x_kvT = nc.declare_dram_parameter("x_kvT", [D, T], BF16, isOutput=False)
    x_qT = nc.declare_dram_parameter("x_qT", [D, NSLOT * P], BF16, isOutput=False)
    w_qkv = nc.declare_dram_parameter("w_qkv", [D, 3 * D], BF16, isOutput=False)
    w_out = nc.declare_dram_parameter("w_out", [D, D], BF16, isOutput=False)
    mask = nc.declare_dram_parameter("mask", [P, NCH, P], BF16, isOutput=False)
    out = nc.declare_dram_parameter("out", [NSLOT * P, D], F32, isOutput=True)

    with tile.TileContext(nc) as tc:
        with tc.tile_pool(name="persist", bufs=1) as pp:
            ident = pp.tile([P, P], BF16)
            make_identity(nc, ident[:])
            kT = pp.tile([P, 8, T], BF16)            # K^T, e-dims on partitions
            qT = pp.tile([P, 8, NSLOT * P], BF16)    # Q^T
            vA = pp.tile([P, NCH, H, DH + 1], BF16)  # V with ones column per head
            yT = pp.tile([P, 8, NSLOT * P], BF16)    # normalized attn out, transposed
            msk = pp.tile([P, NCH, P], BF16)
            nc.gpsimd.memset(vA[:, :, :, DH], 1.0)

            # ---------------- phase 1: QKV projections ------------------------
            with (
                tc.tile_pool(name="ph1", bufs=1) as p1,
                tc.tile_pool(name="wq_pool", bufs=3) as wqp,
                tc.tile_pool(name="wv_pool", bufs=2) as wvp,
                tc.tile_pool(name="ppsum", bufs=4, space="PSUM") as pps,
            ):
                xqTs = p1.tile([P, 8, NSLOT * P], BF16)
                xkTs = p1.tile([P, 8, T], BF16)
                xq_r = x_qT.rearrange("(dc p) t -> p dc t", p=P)
                xk_r = x_kvT.rearrange("(dc p) t -> p dc t", p=P)
                for nt in range(2):
                    nc.sync.dma_start(xqTs[:, :, nt * 512:(nt + 1) * 512],
                                      xq_r[:, :, nt * 512:(nt + 1) * 512])
                for nt in range(4):
                    nc.sync.dma_start(xkTs[:, :, nt * 512:(nt + 1) * 512],
                                      xk_r[:, :, nt * 512:(nt + 1) * 512])

                # Q^T: out[e_tile, q] = sum_d w_q[d, e]^T x_q[d, q]
                for et in range(8):
                    wt = wqp.tile([P, 8, P], BF16, tag="wq")
                    nc.sync.dma_start(
                        wt[:],
                        w_qkv[:, et * P:(et + 1) * P].rearrange("(dc p) e -> p dc e", p=P),
                    )
                    for nt in range(2):
                        ps = pps.tile([P, 512], F32, tag="ps")
                        for dc in range(8):
                            nc.tensor.matmul(
                                ps[:], wt[:, dc, :], xqTs[:, dc, nt * 512:(nt + 1) * 512],
                                start=(dc == 0), stop=(dc == 7),
                            )
                        nc.scalar.copy(out=qT[:, et, nt * 512:(nt + 1) * 512], in_=ps[:])

                # K^T over full 2048 keys
                for et in range(8):
                    wt = wqp.tile([P, 8, P], BF16, tag="wq")
                    nc.sync.dma_start(
                        wt[:],
                        w_qkv[:, D + et * P:D + (et + 1) * P].rearrange("(dc p) e -> p dc e", p=P),
                    )
                    for nt in range(4):
                        ps = pps.tile([P, 512], F32, tag="ps")
                        for dc in range(8):
                            nc.tensor.matmul(
                                ps[:], wt[:, dc, :], xkTs[:, dc, nt * 512:(nt + 1) * 512],
                                start=(dc == 0), stop=(dc == 7),
                            )
                        nc.scalar.copy(out=kT[:, et, nt * 512:(nt + 1) * 512], in_=ps[:])

                # V in natural [t, e] layout, interleaved with the ones column
                for nt in range(2):
                    wv = wvp.tile([P, 8, 512], BF16, tag="wv")
                    nc.sync.dma_start(
                        wv[:],
                        w_qkv[:, 2 * D + nt * 512:2 * D + (nt + 1) * 512].rearrange(
                            "(dc p) e -> p dc e", p=P),
                    )
                    for tt in range(NCH):
                        ps = pps.tile([P, 512], F32, tag="ps")
                        for dc in range(8):
                            nc.tensor.matmul(
                                ps[:], xkTs[:, dc, tt * P:(tt + 1) * P], wv[:, dc, :],
                                start=(dc == 0), stop=(dc == 7),
                            )
                        nc.scalar.copy(
                            out=vA[:, tt, nt * 8:(nt + 1) * 8, 0:DH],
                            in_=ps.rearrange("p (h d) -> p h d", d=DH),
                        )

            nc.sync.dma_start(msk[:], mask[:])

            # ---------------- phase 2: attention ------------------------------
            with (
                tc.tile_pool(name="norm_pool", bufs=4) as np_,
                tc.tile_pool(name="spsum", bufs=4, space="PSUM") as sps,
                tc.tile_pool(name="exp_b", bufs=2) as ep_b,
            ):
                lps = None
                wo = None
                ops = None
                obp = None

                def emit_qk_group(hp, ehs, grp):
                    """One chunk group of QK + exp + mask for head pair hp."""
                    et = hp
                    for r0, eh in ((0, ehs[0]), (64, ehs[1])):
                        lp = lps.tile([P, 1024], F32, tag="lp")
                        pos = 0
                        for c in grp:
                            jm = c // 2
                            ncols = NCOLS[c]
                            s = 0
                            while s < ncols:
                                # a matmul PSUM write must not cross a bank
                                # boundary (512 fp32 per bank)
                                w_ = min(512, ncols - s, 512 - (pos + s) % 512)
                                nc.tensor.matmul(
                                    lp[:, pos + s:pos + s + w_],
                                    kT[r0:r0 + 64, et, c * P:(c + 1) * P],
                                    qT[r0:r0 + 64, et, jm * P + s:jm * P + s + w_],
                                    start=True, stop=True,
                                )
                                s += w_
                            pos += ncols
                        nc.scalar.activation(
                            eh[:, OFF[grp[0]]:OFF[grp[0]] + pos], lp[:, :pos],
                            mybir.ActivationFunctionType.Exp, scale=0.125,
                        )
                        for c in grp:
                            # causal/padding mask on the diagonal slot of chunk c
                            nc.vector.tensor_mul(
                                out=eh[:, OFF[c]:OFF[c] + P],
                                in0=eh[:, OFF[c]:OFF[c] + P],
                                in1=msk[:, c, :],
                            )

                def emit_out_slot(j):
                    op = ops.tile([P, D], F32, tag="op")
                    for nh in range(2):
                        for kc in range(8):
                            nc.tensor.matmul(
                                op[:, nh * 512:(nh + 1) * 512],
                                yT[:, kc, j * P:(j + 1) * P],
                                wo[:, kc, nh * 512:(nh + 1) * 512],
                                start=(kc == 0), stop=(kc == 7),
                            )
                    ob = obp.tile([P, D], F32, tag="ob")
                    nc.vector.tensor_copy(out=ob[:], in_=op[:])
                    nc.sync.dma_start(out[j * P:(j + 1) * P, :], ob[:])

                pending = []   # deferred (et, j, yn2, emit_out) transposes

                def flush_pending():
                    """Transpose+store slots whose normalize finished a while
                    ago - deferring these keeps the PE from stalling on the
                    DVE recip/scale chain right after each slot's AV."""
                    for et, j, yn2, do_out in pending:
                        yt2 = sps.tile([P, P], BF16, tag="small", name=f"yt{et}_{j}")
                        nc.tensor.transpose(yt2[:], yn2[:], ident[:])
                        nc.vector.tensor_copy(out=yT[:, et, j * P:(j + 1) * P], in_=yt2[:])
                        if do_out:
                            emit_out_slot(j)
                    pending.clear()

                def emit_av_slot(hp, ehs, j, emit_out=False):
                    """AV + normalize for slot j of head pair hp."""
                    et = hp
                    yn2 = np_.tile([P, P], BF16, tag="yn2")
                    for parity, eh in enumerate(ehs):
                        h = 2 * hp + parity
                        ya = sps.tile([P, DH + 1], F32, tag="small",
                                      name=f"ya{hp}_{j}_{parity}")
                        for c in range(PL[j]):
                            jm = c // 2
                            nc.tensor.matmul(
                                ya[:],
                                eh[:, OFF[c] + (j - jm) * P:OFF[c] + (j - jm + 1) * P],
                                vA[:, c, h, :],
                                start=(c == 0), stop=(c == PL[j] - 1),
                            )
                        rec = np_.tile([P, 1], F32, tag="rec")
                        nc.vector.reciprocal(rec[:], ya[:, DH:DH + 1])
                        nc.vector.tensor_scalar_mul(
                            yn2[:, parity * DH:(parity + 1) * DH], ya[:, 0:DH], rec[:])
                    flush_pending()
                    pending.append((et, j, yn2, emit_out))

                prev = None
                with (
                    tc.tile_pool(name="exp_a", bufs=2) as ep_a,
                    tc.tile_pool(name="lpsum", bufs=2, space="PSUM") as lps_,
                ):
                    lps = lps_
                    for hp in range(H // 2):
                        pool = ep_a if hp % 2 == 0 else ep_b
                        ehs = (pool.tile([P, OFF[NCH]], BF16, tag="exph", name=f"eh{hp}a"),
                               pool.tile([P, OFF[NCH]], BF16, tag="exph", name=f"eh{hp}b"))
                        # QK stage first (ACT-paced; PE relaxed), then the
                        # previous pair's AV as one dense PE run long enough
                        # to re-warm the HAM clock gate
                        for grp in CGROUPS:
                            emit_qk_group(hp, ehs, grp)
                        if prev is not None:
                            for j in range(NSLOT - 1, -1, -1):
                                emit_av_slot(hp - 1, prev, j)
                        prev = ehs

                # last pair's AV, fused with the output projection
                with (
                    tc.tile_pool(name="wo_pool", bufs=1) as wop,
                    tc.tile_pool(name="ob_pool", bufs=2) as obp_,
                    tc.tile_pool(name="opsum", bufs=2, space="PSUM") as ops_,
                ):
                    obp = obp_
                    ops = ops_
                    wo = wop.tile([P, 8, D], BF16)
                    for dc in range(8):
                        nc.sync.dma_start(wo[:, dc, :], w_out[dc * P:(dc + 1) * P, :])
                    for j in range(NSLOT):
                        emit_av_slot(H // 2 - 1, prev, j, emit_out=True)
                    flush_pending()

    _split_waits(nc, 1)
    return nc


def _split_waits(nc, maxw=1):
    """walrus rejects instructions with more than one sync wait; hoist extra
    waits onto preceding same-engine Drain instructions."""
    nsplit = 0
    for f in nc.m.functions:
        for b in f.blocks:
            insts = b.instructions
            new = []
            changed = False
            for inst in insts:
                si = inst.sync_info
                if si is not None and len(si.on_wait) > maxw:
                    waits = list(si.on_wait)
                    chunks = [waits[i:i + maxw] for i in range(0, len(waits), maxw)]
                    for ci, ch in enumerate(chunks[:-1]):
                        d = mybir.InstDrain(name=f"{inst.name}-wsplit{ci}", ins=[], outs=[])
                        d.engine = inst.engine
                        d.sync_info = mybir.SyncInfo(on_wait=ch, on_update=[])
                        new.append(d)
                        nsplit += 1
                    inst.sync_info = mybir.SyncInfo(
                        on_wait=chunks[-1], on_update=list(si.on_update))
                    changed = True
                new.append(inst)
            if changed:
                b.instructions = new
    return nsplit


def _host_mask(role):
    g = GBLOCKS[role]
    m = np.zeros((P, NCH, P), np.float32)
    for c in range(NCH):
        j = c // 2
        kk = c * P + np.arange(P)[:, None]       # global key index
        qq = g[j] * P + np.arange(P)[None, :]    # global query index
        m[:, c, :] = (kk <= qq).astype(np.float32)
    return m.astype(ml_dtypes.bfloat16)


def kernel(x, w_qkv, w_out):
    global _CACHED_NC, LAST_RESULTS
    x = np.asarray(x)
    w_qkv = np.asarray(w_qkv)
    w_out = np.asarray(w_out)
    B = x.shape[0]
    assert x.shape == (B, T, D) and B * 2 == 8

    if _CACHED_NC is None:
        _CACHED_NC = _build_nc()
    nc = _CACHED_NC

    wq_b = w_qkv.astype(ml_dtypes.bfloat16)
    wo_b = w_out.astype(ml_dtypes.bfloat16)
    masks = {r: _host_mask(r) for r in (0, 1)}

    in_maps = []
    for core in range(8):
        b, role = divmod(core, 2)
        xb = x[b].astype(ml_dtypes.bfloat16)
        g = GBLOCKS[role]
        xq = np.concatenate([xb[gi * P:(gi + 1) * P] for gi in g], axis=0)
        in_maps.append({
            "x_kvT": np.ascontiguousarray(xb.T),
            "x_qT": np.ascontiguousarray(xq.T),
            "w_qkv": wq_b,
            "w_out": wo_b,
            "mask": masks[role],
        })

    res = run_bass_kernel_spmd(nc, in_maps, core_ids=list(range(8)))
    LAST_RESULTS = res

    y = np.empty((B, T, D), np.float32)
    for core in range(8):
        b, role = divmod(core, 2)
        slab = res.results[core]["out"]
        g = GBLOCKS[role]
        for j, gi in enumerate(g):
            y[b, gi * P:(gi + 1) * P, :] = slab[j * P:(j + 1) * P, :]
    return y
